# revision 17
# baseline (speedup 1.0000x reference)
"""Trainium2 Bass kernel for GCNBlock (spectral-norm linear + GCN aggregation +
InstanceNorm + LeakyReLU) distributed across 8 NeuronCores.

Strategy (per the dst-sharding hint):
  - out = A @ (x @ WnT) = (A @ x) @ WnT, where A is the symmetric-normalized
    adjacency (with self loops).  We aggregate raw x rows first, then apply the
    128x128 weight per dst tile -- no h materialization.
  - Normalization factorization: host pre-scales rows xs = dinv[src] * x.  The
    remaining per-dst scale dinv[dst] is a positive per-row factor, which
    cancels inside InstanceNorm's (z - mu)/sigma EXCEPT through the bias and
    eps; both are fixed exactly by using bias' = sqrt(deg)*b (folded into the
    bias matmul's lhsT) and eps' = deg*eps (per-row eps).  This makes every
    scatter matrix S a PURE one-hot (no coef multiply), and all self-loop
    blocks share a single constant identity S.
  - dst nodes sharded across 8 cores (6272 nodes = 49 tiles of 128 per core).
    Edges partitioned by dst on the host, sorted into per-(tile, src-half)
    groups padded to 128-edge blocks (src halves because dma_gather indices
    are int16).  Pad slots carry dstloc=-1 so their S row is all-zero.
  - Per block: bulk-gather 128 src rows of xs (bf16) via dma_gather and
    accumulate aggT[cin, dst] += Xsrc.T @ S on the PE in PSUM.  The one-hot
    scatter matrices S are PRECOMPUTED ON THE HOST (exact 0/1 in bf16) and
    streamed per chunk via a single line-rate HWDGE DMA ([e, col, dst]-major
    layout -> one contiguous descriptor per partition).  This keeps the hot
    loop entirely free of DVE work: the DVE fully blocks against SWDGE
    descriptor generation (exclusive shared SBUF port pair with GpSimd), so
    any in-loop DVE op would serialize against the gathers.
  - Self-loop rows are NOT gathered: each core's own xs slice arrives as a
    separate sequential input (xself) DMA'd per tile into a dedicated "self"
    block using the shared identity S.
  - dma_gather descriptor generation (Q7 SWDGE) is the main bottleneck at
    ~8-10ns/index per Q7 core pair.  num_swdge_queues=4 + round-robin
    queue_num spreads desc-gen over all 4 Q7 core pairs concurrently.
  - To keep the hot loop free of po-dependent ops (which would convoy the
    in-order DVE/ACT queues), the main stream does ONLY gathers + S-builds +
    aggregation matmuls + one PSUM->SBUF copy per tile into a persistent
    agg_all buffer.  The weight matmul (with a 129th column computing the row
    mean for free) runs in a tail phase and is executed TWICE on the
    otherwise-idle PE: once to collect mean (ACT copy) and sum(po^2) (ACT
    Square accum_out), and -- after a handful of BATCHED [P,49] DVE ops
    compute rstd and -mu*rstd -- a second time feeding one fused ACT
    Prelu(po*rstd - mu*rstd, alpha=0.2) per tile straight from PSUM, then the
    output DMA.
"""

import numpy as np
import ml_dtypes
from contextlib import ExitStack

import concourse.tile as tile
from concourse import bacc, mybir
from concourse.bass_utils import run_bass_kernel_spmd

# Problem constants (hardcoded per spec)
N, E, C = 50000, 800000, 128
P = 128
NCORES = 8
TPC = 49                # dst tiles per core
NPC = TPC * P           # 6272 dst nodes per core
NPAD = NCORES * NPC     # 50176 padded node count
HALF = 32768            # int16 index split point
CHUNK_TILES = 3
NCHUNKS = -(-TPC // CHUNK_TILES)  # 17 (last chunk ragged)
ACT_FRAC = 5            # every 5th S-build goes to the Scalar engine
NQ = 4                  # SWDGE queues (Q7 core pairs) used for gathers
EPS_IN = 1e-5
CW = C + 1              # weight matmul width (extra column = row mean)


def _preprocess(x, edge_index, W, b, u):
    """Host-side prep: spectral norm, edge partitioning, metadata layout."""
    x = np.asarray(x, dtype=np.float32)
    ei = np.asarray(edge_index)
    W = np.asarray(W, dtype=np.float32)
    b = np.asarray(b, dtype=np.float32)
    u = np.asarray(u, dtype=np.float32)

    # --- spectral norm (one power iteration), matches reference ---
    eps = np.float32(1e-12)
    v = (W.T @ u).astype(np.float32)
    v = v / (np.float32(np.linalg.norm(v)) + eps)
    Wv = (W @ v).astype(np.float32)
    u2 = Wv / (np.float32(np.linalg.norm(Wv)) + eps)
    sigma = np.float32(u2 @ Wv)
    WnT = np.ascontiguousarray((W / sigma).T, dtype=np.float32)  # [cin, cout]

    src = ei[0].astype(np.int64)
    dst = ei[1].astype(np.int64)

    # --- degrees; xs = dinv * x (row-scaled source features) ---
    deg = (np.bincount(dst, minlength=N) + 1).astype(np.float32)
    dinv = (1.0 / np.sqrt(deg)).astype(np.float32)
    deg_pad = np.ones(NPAD, dtype=np.float32)
    deg_pad[:N] = deg
    sqrtdeg_pad = np.sqrt(deg_pad)

    # --- group real edges by (core, tile, src-half) ---
    core = dst // NPC
    tile_g = (dst % NPC) // P
    dstloc = (dst % P).astype(np.float32)
    half = (src >= HALF).astype(np.int64)
    key = ((core * TPC + tile_g) * 2 + half).astype(np.int64)
    NG = NCORES * TPC * 2
    order = np.argsort(key, kind="stable")
    counts = np.bincount(key, minlength=NG)
    starts = np.zeros(NG + 1, dtype=np.int64)
    np.cumsum(counts, out=starts[1:])
    rank = np.arange(len(key), dtype=np.int64) - starts[key[order]]

    cnt3 = counts.reshape(NCORES, TPC, 2)
    nb = np.ceil(cnt3.max(axis=0) / P).astype(np.int64)  # [TPC, 2] gather blocks

    # Column layout per chunk: [self cols t0..t1) | lo blocks | hi blocks].
    # Gather-only column index (gcol) is separate and skips self columns.
    col_of_self = np.zeros(TPC, dtype=np.int64)
    blk_col = np.zeros((TPC, 2), dtype=np.int64)   # global gat column of block run
    blk_gcol = np.zeros((TPC, 2), dtype=np.int64)  # global gather column of run
    gather_gcol0 = np.zeros((NCHUNKS, 2), dtype=np.int64)
    gather_nblk = np.zeros((NCHUNKS, 2), dtype=np.int64)
    chunk_col0 = np.zeros(NCHUNKS, dtype=np.int64)
    cpos = 0
    gpos = 0
    for ci in range(NCHUNKS):
        t0 = ci * CHUNK_TILES
        t1 = min(t0 + CHUNK_TILES, TPC)
        chunk_col0[ci] = cpos
        for t in range(t0, t1):
            col_of_self[t] = cpos
            cpos += 1
        for h in range(2):
            gather_gcol0[ci, h] = gpos
            for t in range(t0, t1):
                blk_col[t, h] = cpos
                blk_gcol[t, h] = gpos
                cpos += nb[t, h]
                gpos += nb[t, h]
            gather_nblk[ci, h] = gpos - gather_gcol0[ci, h]
    totcol = cpos
    totg = gpos

    # host-precomputed one-hot scatter blocks S[e, col, dst] (exact 0/1 bf16);
    # [e, col, dst]-major so a chunk's S loads as one descriptor per partition
    SBLK = np.zeros((NCORES, P, totcol, P), dtype=ml_dtypes.bfloat16)
    IDXALL = np.zeros((NCORES, totg * P), dtype=np.int16)

    o_core = core[order]
    o_tile = tile_g[order]
    o_half = half[order]
    o_col = blk_col[o_tile, o_half] + rank // P
    o_gcol = blk_gcol[o_tile, o_half] + rank // P
    o_slot = rank % P

    SBLK[o_core, o_slot, o_col, dstloc[order].astype(np.int64)] = 1.0
    arangeP_i = np.arange(P)
    for t in range(TPC):
        SBLK[:, arangeP_i, col_of_self[t], arangeP_i] = 1.0
    IDXALL[o_core, o_gcol * P + o_slot] = (src[order] - o_half * HALF).astype(np.int16)

    # idx SBUF layout: pos k -> [k % 16, k // 16], replicated 8x over partitions
    IDX = np.tile(IDXALL.reshape(NCORES, -1, 16).transpose(0, 2, 1), (1, 8, 1))

    xs_pad = np.zeros((NPAD, C), dtype=ml_dtypes.bfloat16)
    xs_pad[:N] = (dinv[:, None] * x).astype(ml_dtypes.bfloat16)
    XSELF = xs_pad.reshape(NCORES, TPC, P, C)

    SQRTDEG = sqrtdeg_pad.reshape(NCORES, 1, NPC).astype(np.float32)
    EPSDEG = (EPS_IN * deg_pad).reshape(NCORES, TPC, P).transpose(0, 2, 1)
    EPSDEG = np.ascontiguousarray(EPSDEG, dtype=np.float32)  # [NCORES, P, TPC]

    meta = dict(
        nb=nb,
        col_of_self=col_of_self,
        blk_col=blk_col,
        blk_gcol=blk_gcol,
        chunk_col0=chunk_col0,
        gather_gcol0=gather_gcol0,
        gather_nblk=gather_nblk,
        totcol=totcol,
        totg=totg,
    )
    return xs_pad, XSELF, IDX, SBLK, SQRTDEG, EPSDEG, WnT, b.reshape(1, C).astype(np.float32), meta


def _build(meta):
    """Build the SPMD Bass graph (shared across all 8 cores)."""
    nb = meta["nb"]
    col_of_self = meta["col_of_self"]
    blk_col = meta["blk_col"]
    blk_gcol = meta["blk_gcol"]
    chunk_col0 = meta["chunk_col0"]
    gather_gcol0 = meta["gather_gcol0"]
    gather_nblk = meta["gather_nblk"]
    totcol = meta["totcol"]
    totg = meta["totg"]

    nc = bacc.Bacc(
        "TRN2", target_bir_lowering=False, debug=False, num_swdge_queues=NQ
    )

    x_d = nc.dram_tensor("x", [NPAD, C], mybir.dt.bfloat16, kind="ExternalInput")
    xself_d = nc.dram_tensor("xself", [TPC, P, C], mybir.dt.bfloat16, kind="ExternalInput")
    idx_d = nc.dram_tensor("idx", [P, totg * 8], mybir.dt.int16, kind="ExternalInput")
    s_d = nc.dram_tensor("s", [P, totcol * P], mybir.dt.bfloat16, kind="ExternalInput")
    sd_d = nc.dram_tensor("sqrtdeg", [1, NPC], mybir.dt.float32, kind="ExternalInput")
    epsdeg_d = nc.dram_tensor("epsdeg", [P, TPC], mybir.dt.float32, kind="ExternalInput")
    wnT_d = nc.dram_tensor("wnT", [C, C], mybir.dt.float32, kind="ExternalInput")
    b_d = nc.dram_tensor("b", [1, C], mybir.dt.float32, kind="ExternalInput")
    out_d = nc.dram_tensor("out", [NPC, C], mybir.dt.float32, kind="ExternalOutput")

    # max gather blocks per chunk for each half (separate tiles per half)
    nlo_max = max(int(gather_nblk[ci, 0]) for ci in range(NCHUNKS))
    nhi_max = max(int(gather_nblk[ci, 1]) for ci in range(NCHUNKS))

    qctr = 0  # gather round-robin queue counter

    with tile.TileContext(nc) as tc, ExitStack() as ctx:
        meta_p = ctx.enter_context(tc.tile_pool(name="meta", bufs=1))
        gat_p = ctx.enter_context(tc.tile_pool(name="gat", bufs=5))
        s_p = ctx.enter_context(tc.tile_pool(name="s", bufs=4))
        agg_p = ctx.enter_context(tc.tile_pool(name="agg", bufs=4))
        out_p = ctx.enter_context(tc.tile_pool(name="out", bufs=6))
        small_p = ctx.enter_context(tc.tile_pool(name="small", bufs=1))
        ps_p = ctx.enter_context(tc.tile_pool(name="ps", bufs=8, space="PSUM"))

        idxz = meta_p.tile([P, 8], mybir.dt.int16)
        nc.vector.memset(idxz[:], 0)
        warm = meta_p.tile([P, 1, P], mybir.dt.bfloat16)
        nc.gpsimd.dma_gather(
            out_ap=warm[:], in_ap=x_d[0:HALF, :], idxs_ap=idxz[:, 0:8],
            num_idxs=P, num_idxs_reg=P, elem_size=C,
            single_packet=False, queue_num=0,
        )
        idx_sb = meta_p.tile([P, totg * 8], mybir.dt.int16)
        nc.sync.dma_start(idx_sb[:], idx_d[:])
        sd_sb = meta_p.tile([1, NPC], mybir.dt.float32)
        nc.sync.dma_start(sd_sb[:], sd_d[:])
        epsdeg_sb = meta_p.tile([P, TPC], mybir.dt.float32)
        nc.sync.dma_start(epsdeg_sb[:], epsdeg_d[:])
        wnT_sb = meta_p.tile([C, C], mybir.dt.float32)
        nc.sync.dma_start(wnT_sb[:], wnT_d[:])
        b_sb = meta_p.tile([1, C], mybir.dt.float32)
        nc.sync.dma_start(b_sb[:], b_d[:])

        # persistent aggregation output + norm-stat staging for the tail
        agg_all = meta_p.tile([P, TPC * C], mybir.dt.float32)
        mv_all = meta_p.tile([P, 2 * TPC], mybir.dt.float32)

        x_lo = x_d[0:HALF, :]
        x_hi = x_d[HALF:NPAD, :]

        for ci in range(NCHUNKS):
            t0 = ci * CHUNK_TILES
            t1 = min(t0 + CHUNK_TILES, TPC)
            nt = t1 - t0
            ccol0 = int(chunk_col0[ci])
            ncols = nt + int(gather_nblk[ci].sum())
            s_sb = s_p.tile([P, (CHUNK_TILES + nlo_max + nhi_max) * P], mybir.dt.bfloat16, tag="sblk")
            nc.sync.dma_start(
                s_sb[:, 0 : ncols * P], s_d[:, ccol0 * P : (ccol0 + ncols) * P]
            )
            gat_self = gat_p.tile([P, CHUNK_TILES, P], mybir.dt.bfloat16, tag="gself")
            gat_lo = gat_p.tile([P, nlo_max, P], mybir.dt.bfloat16, tag="glo")
            gat_hi = gat_p.tile([P, nhi_max, P], mybir.dt.bfloat16, tag="ghi")
            gat_half = [gat_lo, gat_hi]

            # self blocks: sequential HWDGE loads of this core's own xs rows
            for t in range(t0, t1):
                nc.sync.dma_start(gat_self[:, t - t0, :], xself_d[t])

            # gathers: separate dst tiles per half -> no WAW between them, so
            # up to 4 gathers (2 chunks x 2 halves) run on 4 Q7 pairs at once.
            # Alternate half order per chunk so queue round-robin spreads the
            # (larger) lo and (smaller) hi calls evenly.
            halves = ((0, x_lo), (1, x_hi)) if ci % 2 == 0 else ((1, x_hi), (0, x_lo))
            for h, src_ap in halves:
                nblk_g = int(gather_nblk[ci, h])
                if nblk_g == 0:
                    continue
                # split into two sub-gathers on different queues; rotate the
                # queue offset per chunk so lo/hi sizes balance across queues
                nb1 = (nblk_g + 1) // 2
                for b0, b1 in ((0, nb1), (nb1, nblk_g)):
                    if b1 <= b0:
                        continue
                    nidx = (b1 - b0) * P
                    ic0 = (int(gather_gcol0[ci, h]) + b0) * 8
                    nc.gpsimd.dma_gather(
                        out_ap=gat_half[h][:, b0:b1, :],
                        in_ap=src_ap,
                        idxs_ap=idx_sb[:, ic0 : ic0 + nidx // 16],
                        num_idxs=nidx,
                        num_idxs_reg=nidx,
                        elem_size=C,
                        single_packet=False,
                        queue_num=(qctr + ci) % NQ,
                    )
                    qctr += 1

            for t in range(t0, t1):
                # (S column in s_sb, source tile, source column) per block
                blocks = [(int(col_of_self[t]) - ccol0, gat_self, t - t0)]
                for h in range(2):
                    loc0 = int(blk_gcol[t, h]) - int(gather_gcol0[ci, h])
                    for j in range(int(nb[t, h])):
                        blocks.append(
                            (int(blk_col[t, h]) + j - ccol0, gat_half[h], loc0 + j)
                        )

                pt = ps_p.tile([P, C], mybir.dt.float32, tag="ps")
                for j, (scol, gtile, gcol) in enumerate(blocks):
                    nc.tensor.matmul(
                        pt[:],
                        lhsT=gtile[:, gcol, :],
                        rhs=s_sb[:, scol * P : (scol + 1) * P],
                        start=(j == 0),
                        stop=(j == len(blocks) - 1),
                    )

                nc.scalar.copy(agg_all[:, t * C : (t + 1) * C], pt[:])

        # ---- tail pass 1 (post-gathers: DVE is free once SWDGE desc-gen ends).
        # Gate pass-1 matmuls on the last aggcopy so their PSUM slots never pin
        # while gathers still run (bn_stats on DVE would block until then).
        zc1 = small_p.tile([P, 1], mybir.dt.float32, tag="zc1")
        nc.vector.tensor_scalar(
            out=zc1[:], in0=agg_all[:, (TPC - 1) * C : (TPC - 1) * C + 1],
            scalar1=0.0, scalar2=None, op0=mybir.AluOpType.mult,
        )
        wnT1_sb = meta_p.tile([C, C], mybir.dt.float32)
        nc.scalar.activation(
            out=wnT1_sb[:], in_=wnT_sb[:],
            func=mybir.ActivationFunctionType.Identity, bias=zc1[0:C, 0:1], scale=1.0,
        )
        for t in range(TPC):
            po = ps_p.tile([P, C], mybir.dt.float32, tag="ps")
            nc.tensor.matmul(
                po[:], lhsT=agg_all[:, t * C : (t + 1) * C], rhs=wnT1_sb[:],
                start=True, stop=False,
            )
            # bias' = sqrt(deg) * b (per-dst row scale folded into lhsT)
            nc.tensor.matmul(
                po[:], lhsT=sd_sb[:, t * P : (t + 1) * P], rhs=b_sb[:],
                start=False, stop=True,
            )
            stats = small_p.tile([P, 6], mybir.dt.float32, tag="stats")
            nc.vector.bn_stats(out=stats[:], in_=po[:])
            nc.vector.bn_aggr(out=mv_all[:, 2 * t : 2 * t + 2], in_=stats[:])

        # ---- batched norm: all [P,1]-per-tile scalars as [P,TPC] ops ----
        mu_ap = mv_all[:, 0 : 2 * TPC : 2]
        var_ap = mv_all[:, 1 : 2 * TPC : 2]
        var3 = small_p.tile([P, TPC], mybir.dt.float32, tag="var3")
        nc.vector.tensor_tensor(out=var3[:], in0=var_ap, in1=epsdeg_sb[:], op=mybir.AluOpType.add)
        std = small_p.tile([P, TPC], mybir.dt.float32, tag="std")
        nc.scalar.activation(
            out=std[:], in_=var3[:], func=mybir.ActivationFunctionType.Sqrt,
        )
        rstd = small_p.tile([P, TPC], mybir.dt.float32, tag="rstd")
        nc.vector.reciprocal(out=rstd[:], in_=std[:])
        nmr0 = small_p.tile([P, TPC], mybir.dt.float32, tag="nmr0")
        nc.vector.tensor_tensor(out=nmr0[:], in0=mu_ap, in1=rstd[:], op=mybir.AluOpType.mult)
        nmr = small_p.tile([P, TPC], mybir.dt.float32, tag="nmr")
        nc.vector.tensor_scalar(
            out=nmr[:], in0=nmr0[:], scalar1=-1.0, scalar2=None,
            op0=mybir.AluOpType.mult,
        )

        # ---- tail pass 2: recompute po on the idle PE, fused norm+LeakyReLU ----
        # wnT2 is written only after the batched norm, so pass-2 matmuls cannot
        # be scheduled early (their PSUM tiles would pin slots until Prelu).
        zcol = small_p.tile([P, 1], mybir.dt.float32, tag="zcol")
        nc.vector.tensor_scalar(
            out=zcol[:], in0=rstd[:, 0:1], scalar1=0.0, scalar2=None,
            op0=mybir.AluOpType.mult,
        )
        wnT2_sb = meta_p.tile([C, C], mybir.dt.float32)
        nc.scalar.activation(
            out=wnT2_sb[:], in_=wnT_sb[:],
            func=mybir.ActivationFunctionType.Identity, bias=zcol[0:C, 0:1], scale=1.0,
        )
        for t in range(TPC):
            po2 = ps_p.tile([P, C], mybir.dt.float32, tag="ps")
            nc.tensor.matmul(
                po2[:], lhsT=agg_all[:, t * C : (t + 1) * C], rhs=wnT2_sb[:],
                start=True, stop=False,
            )
            nc.tensor.matmul(
                po2[:], lhsT=sd_sb[:, t * P : (t + 1) * P], rhs=b_sb[:],
                start=False, stop=True,
            )
            final = out_p.tile([P, P], mybir.dt.float32, tag="final")
            # fused normalize + LeakyReLU: Prelu(po*rstd - mu*rstd, alpha=0.2)
            nc.scalar.activation(
                out=final[:], in_=po2[:],
                func=mybir.ActivationFunctionType.Prelu,
                bias=nmr[:, t : t + 1], scale=rstd[:, t : t + 1], alpha=0.2,
            )
            nc.sync.dma_start(out_d[t * P : (t + 1) * P, :], final[:])

    nc.compile()
    return nc


def _make_in_maps(xs_pad, XSELF, IDX, SBLK, SQRTDEG, EPSDEG, WnT, bvec):
    return [
        {
            "x": xs_pad,
            "xself": np.ascontiguousarray(XSELF[i]),
            "idx": np.ascontiguousarray(IDX[i]),
            "s": np.ascontiguousarray(SBLK[i].reshape(P, -1)),
            "sqrtdeg": np.ascontiguousarray(SQRTDEG[i]),
            "epsdeg": np.ascontiguousarray(EPSDEG[i]),
            "wnT": WnT,
            "b": bvec,
        }
        for i in range(NCORES)
    ]


def kernel(x, edge_index, W, b, u):
    pre = _preprocess(x, edge_index, W, b, u)
    nc = _build(pre[-1])
    in_maps = _make_in_maps(*pre[:-1])

    # The axon terminal can be transiently unavailable right after a prior
    # process's teardown; retry with backoff.
    import time

    last_err = None
    for attempt in range(6):
        try:
            res = run_bass_kernel_spmd(nc, in_maps, list(range(NCORES)))
            break
        except Exception as e:  # noqa: BLE001
            last_err = e
            time.sleep(45)
    else:
        raise last_err
    shards = [np.asarray(res.results[i]["out"]) for i in range(NCORES)]
    out = np.concatenate(shards, axis=0)[:N]
    return out.astype(np.float32)


# revision 18
# speedup vs baseline: 1.1885x; 1.1885x over previous
"""Trainium2 Bass kernel for GCNBlock (spectral-norm linear + GCN aggregation +
InstanceNorm + LeakyReLU) distributed across 8 NeuronCores.

Strategy (per the dst-sharding hint):
  - out = A @ (x @ WnT) = (A @ x) @ WnT, where A is the symmetric-normalized
    adjacency (with self loops).  We aggregate raw x rows first, then apply the
    128x128 weight per dst tile -- no h materialization.
  - Normalization factorization: host pre-scales rows xs = dinv[src] * x.  The
    remaining per-dst scale dinv[dst] is a positive per-row factor, which
    cancels inside InstanceNorm's (z - mu)/sigma EXCEPT through the bias and
    eps; both are fixed exactly by using bias' = sqrt(deg)*b (folded into the
    bias matmul's lhsT) and eps' = deg*eps (per-row eps).  This makes every
    scatter matrix S a PURE one-hot (no coef multiply), and all self-loop
    blocks share a single constant identity S.
  - dst nodes sharded across 8 cores (6272 nodes = 49 tiles of 128 per core).
    Edges partitioned by dst on the host, sorted into per-(tile, src-half)
    groups padded to 128-edge blocks (src halves because dma_gather indices
    are int16).  Pad slots carry dstloc=-1 so their S row is all-zero.
  - Per block: bulk-gather 128 src rows of xs (bf16) via dma_gather and
    accumulate aggT[cin, dst] += Xsrc.T @ S on the PE in PSUM.  The one-hot
    scatter matrices S are PRECOMPUTED ON THE HOST (exact 0/1 in bf16) and
    streamed per chunk via a single line-rate HWDGE DMA ([e, col, dst]-major
    layout -> one contiguous descriptor per partition).  This keeps the hot
    loop entirely free of DVE work: the DVE fully blocks against SWDGE
    descriptor generation (exclusive shared SBUF port pair with GpSimd), so
    any in-loop DVE op would serialize against the gathers.
  - Self-loop rows are NOT gathered: each core's own xs slice arrives as a
    separate sequential input (xself) DMA'd per tile into a dedicated "self"
    block using the shared identity S.
  - dma_gather descriptor generation (Q7 SWDGE) is the main bottleneck at
    ~8-10ns/index per Q7 core pair.  num_swdge_queues=4 + round-robin
    queue_num spreads desc-gen over all 4 Q7 core pairs concurrently.
  - To keep the hot loop free of po-dependent ops (which would convoy the
    in-order DVE/ACT queues), the main stream does ONLY gathers + S-builds +
    aggregation matmuls + one PSUM->SBUF copy per tile into a persistent
    agg_all buffer.  The weight matmul (with a 129th column computing the row
    mean for free) runs in a tail phase and is executed TWICE on the
    otherwise-idle PE: once to collect mean (ACT copy) and sum(po^2) (ACT
    Square accum_out), and -- after a handful of BATCHED [P,49] DVE ops
    compute rstd and -mu*rstd -- a second time feeding one fused ACT
    Prelu(po*rstd - mu*rstd, alpha=0.2) per tile straight from PSUM, then the
    output DMA.
"""

import numpy as np
import ml_dtypes
from contextlib import ExitStack

import concourse.tile as tile
from concourse import bacc, mybir
from concourse.bass_utils import run_bass_kernel_spmd

# Problem constants (hardcoded per spec)
N, E, C = 50000, 800000, 128
P = 128
NCORES = 8
TPC = 49                # dst tiles per core
NPC = TPC * P           # 6272 dst nodes per core
NPAD = NCORES * NPC     # 50176 padded node count
HALF = 32768            # int16 index split point
CHUNK_TILES = 3
NCHUNKS = -(-TPC // CHUNK_TILES)  # 17 (last chunk ragged)
ACT_FRAC = 5            # every 5th S-build goes to the Scalar engine
NQ = 4                  # SWDGE queues (Q7 core pairs) used for gathers
EPS_IN = 1e-5
CW = C + 1              # weight matmul width (extra column = row mean)


def _preprocess(x, edge_index, W, b, u):
    """Host-side prep: spectral norm, edge partitioning, metadata layout."""
    x = np.asarray(x, dtype=np.float32)
    ei = np.asarray(edge_index)
    W = np.asarray(W, dtype=np.float32)
    b = np.asarray(b, dtype=np.float32)
    u = np.asarray(u, dtype=np.float32)

    # --- spectral norm (one power iteration), matches reference ---
    eps = np.float32(1e-12)
    v = (W.T @ u).astype(np.float32)
    v = v / (np.float32(np.linalg.norm(v)) + eps)
    Wv = (W @ v).astype(np.float32)
    u2 = Wv / (np.float32(np.linalg.norm(Wv)) + eps)
    sigma = np.float32(u2 @ Wv)
    WnT = np.ascontiguousarray((W / sigma).T, dtype=np.float32)  # [cin, cout]
    # extended weight: col C = row-mean column (mu comes out of the matmul)
    WnT_ext = np.concatenate([WnT, WnT.mean(axis=1, keepdims=True)], axis=1)
    WnT_ext = np.ascontiguousarray(WnT_ext, dtype=np.float32)
    b_ext = np.concatenate([b, [b.mean()]]).reshape(1, CW).astype(np.float32)
    b_nonzero = bool(np.any(b))

    src = ei[0].astype(np.int64)
    dst = ei[1].astype(np.int64)

    # --- degrees; xs = dinv * x (row-scaled source features) ---
    deg = (np.bincount(dst, minlength=N) + 1).astype(np.float32)
    dinv = (1.0 / np.sqrt(deg)).astype(np.float32)
    deg_pad = np.ones(NPAD, dtype=np.float32)
    deg_pad[:N] = deg
    sqrtdeg_pad = np.sqrt(deg_pad)

    # --- group real edges by (core, tile, src-half) ---
    core = dst // NPC
    tile_g = (dst % NPC) // P
    dstloc = (dst % P).astype(np.float32)
    half = (src >= HALF).astype(np.int64)
    key = ((core * TPC + tile_g) * 2 + half).astype(np.int64)
    NG = NCORES * TPC * 2
    order = np.argsort(key, kind="stable")
    counts = np.bincount(key, minlength=NG)
    starts = np.zeros(NG + 1, dtype=np.int64)
    np.cumsum(counts, out=starts[1:])
    rank = np.arange(len(key), dtype=np.int64) - starts[key[order]]

    cnt3 = counts.reshape(NCORES, TPC, 2)
    nb = np.ceil(cnt3.max(axis=0) / P).astype(np.int64)  # [TPC, 2] gather blocks

    # Column layout per chunk: [self cols t0..t1) | lo blocks | hi blocks].
    # Gather-only column index (gcol) is separate and skips self columns.
    col_of_self = np.zeros(TPC, dtype=np.int64)
    blk_col = np.zeros((TPC, 2), dtype=np.int64)   # global gat column of block run
    blk_gcol = np.zeros((TPC, 2), dtype=np.int64)  # global gather column of run
    gather_gcol0 = np.zeros((NCHUNKS, 2), dtype=np.int64)
    gather_nblk = np.zeros((NCHUNKS, 2), dtype=np.int64)
    chunk_col0 = np.zeros(NCHUNKS, dtype=np.int64)
    cpos = 0
    gpos = 0
    for ci in range(NCHUNKS):
        t0 = ci * CHUNK_TILES
        t1 = min(t0 + CHUNK_TILES, TPC)
        chunk_col0[ci] = cpos
        for t in range(t0, t1):
            col_of_self[t] = cpos
            cpos += 1
        for h in range(2):
            gather_gcol0[ci, h] = gpos
            for t in range(t0, t1):
                blk_col[t, h] = cpos
                blk_gcol[t, h] = gpos
                cpos += nb[t, h]
                gpos += nb[t, h]
            gather_nblk[ci, h] = gpos - gather_gcol0[ci, h]
    totcol = cpos
    totg = gpos

    # host-precomputed one-hot scatter blocks S[e, col, dst] (exact 0/1 bf16);
    # [e, col, dst]-major so a chunk's S loads as one descriptor per partition
    SBLK = np.zeros((NCORES, P, totcol, P), dtype=ml_dtypes.bfloat16)
    IDXALL = np.zeros((NCORES, totg * P), dtype=np.int16)

    o_core = core[order]
    o_tile = tile_g[order]
    o_half = half[order]
    o_col = blk_col[o_tile, o_half] + rank // P
    o_gcol = blk_gcol[o_tile, o_half] + rank // P
    o_slot = rank % P

    SBLK[o_core, o_slot, o_col, dstloc[order].astype(np.int64)] = 1.0
    arangeP_i = np.arange(P)
    for t in range(TPC):
        SBLK[:, arangeP_i, col_of_self[t], arangeP_i] = 1.0
    IDXALL[o_core, o_gcol * P + o_slot] = (src[order] - o_half * HALF).astype(np.int16)

    # idx SBUF layout: pos k -> [k % 16, k // 16], replicated 8x over partitions
    IDX = np.tile(IDXALL.reshape(NCORES, -1, 16).transpose(0, 2, 1), (1, 8, 1))

    xs_pad = np.zeros((NPAD, C), dtype=ml_dtypes.bfloat16)
    xs_pad[:N] = (dinv[:, None] * x).astype(ml_dtypes.bfloat16)
    XSELF = xs_pad.reshape(NCORES, TPC, P, C)

    SQRTDEG = sqrtdeg_pad.reshape(NCORES, 1, NPC).astype(np.float32)
    EPSDEG = (EPS_IN * deg_pad).reshape(NCORES, TPC, P).transpose(0, 2, 1)
    EPSDEG = np.ascontiguousarray(EPSDEG, dtype=np.float32)  # [NCORES, P, TPC]

    meta = dict(
        nb=nb,
        col_of_self=col_of_self,
        blk_col=blk_col,
        blk_gcol=blk_gcol,
        chunk_col0=chunk_col0,
        gather_gcol0=gather_gcol0,
        gather_nblk=gather_nblk,
        totcol=totcol,
        totg=totg,
    )
    meta['b_nonzero'] = b_nonzero
    return xs_pad, XSELF, IDX, SBLK, SQRTDEG, EPSDEG, WnT_ext, b_ext, meta


def _build(meta):
    """Build the SPMD Bass graph (shared across all 8 cores)."""
    nb = meta["nb"]
    b_nonzero = meta["b_nonzero"]
    col_of_self = meta["col_of_self"]
    blk_col = meta["blk_col"]
    blk_gcol = meta["blk_gcol"]
    chunk_col0 = meta["chunk_col0"]
    gather_gcol0 = meta["gather_gcol0"]
    gather_nblk = meta["gather_nblk"]
    totcol = meta["totcol"]
    totg = meta["totg"]

    nc = bacc.Bacc(
        "TRN2", target_bir_lowering=False, debug=False, num_swdge_queues=NQ
    )

    x_d = nc.dram_tensor("x", [NPAD, C], mybir.dt.bfloat16, kind="ExternalInput")
    xself_d = nc.dram_tensor("xself", [TPC, P, C], mybir.dt.bfloat16, kind="ExternalInput")
    idx_d = nc.dram_tensor("idx", [P, totg * 8], mybir.dt.int16, kind="ExternalInput")
    s_d = nc.dram_tensor("s", [P, totcol * P], mybir.dt.bfloat16, kind="ExternalInput")
    sd_d = nc.dram_tensor("sqrtdeg", [1, NPC], mybir.dt.float32, kind="ExternalInput")
    epsdeg_d = nc.dram_tensor("epsdeg", [P, TPC], mybir.dt.float32, kind="ExternalInput")
    wnT_d = nc.dram_tensor("wnT", [C, CW], mybir.dt.float32, kind="ExternalInput")
    b_d = nc.dram_tensor("b", [1, CW], mybir.dt.float32, kind="ExternalInput")
    out_d = nc.dram_tensor("out", [NPC, C], mybir.dt.float32, kind="ExternalOutput")

    # max gather blocks per chunk for each half (separate tiles per half)
    nlo_max = max(int(gather_nblk[ci, 0]) for ci in range(NCHUNKS))
    nhi_max = max(int(gather_nblk[ci, 1]) for ci in range(NCHUNKS))

    qctr = 0  # gather round-robin queue counter

    with tile.TileContext(nc) as tc, ExitStack() as ctx:
        meta_p = ctx.enter_context(tc.tile_pool(name="meta", bufs=1))
        gat_p = ctx.enter_context(tc.tile_pool(name="gat", bufs=5))
        s_p = ctx.enter_context(tc.tile_pool(name="s", bufs=4))
        agg_p = ctx.enter_context(tc.tile_pool(name="agg", bufs=4))
        out_p = ctx.enter_context(tc.tile_pool(name="out", bufs=6))
        small_p = ctx.enter_context(tc.tile_pool(name="small", bufs=1))
        ps_p = ctx.enter_context(tc.tile_pool(name="ps", bufs=8, space="PSUM"))

        idxz = meta_p.tile([P, 8], mybir.dt.int16)
        nc.vector.memset(idxz[:], 0)
        warm = meta_p.tile([P, 1, P], mybir.dt.bfloat16)
        nc.gpsimd.dma_gather(
            out_ap=warm[:], in_ap=x_d[0:HALF, :], idxs_ap=idxz[:, 0:8],
            num_idxs=P, num_idxs_reg=P, elem_size=C,
            single_packet=False, queue_num=0,
        )
        idx_sb = meta_p.tile([P, totg * 8], mybir.dt.int16)
        nc.sync.dma_start(idx_sb[:], idx_d[:])
        sd_sb = meta_p.tile([1, NPC], mybir.dt.float32)
        nc.sync.dma_start(sd_sb[:], sd_d[:])
        epsdeg_sb = meta_p.tile([P, TPC], mybir.dt.float32)
        nc.sync.dma_start(epsdeg_sb[:], epsdeg_d[:])
        wnT_sb = meta_p.tile([C, CW], mybir.dt.float32)
        nc.sync.dma_start(wnT_sb[:], wnT_d[:])
        b_sb = meta_p.tile([1, CW], mybir.dt.float32)
        nc.sync.dma_start(b_sb[:], b_d[:])

        # persistent aggregation output + norm-stat staging for the tail
        agg_all = meta_p.tile([P, TPC * C], mybir.dt.float32)
        mu_sb = meta_p.tile([P, TPC], mybir.dt.float32)
        ssq_sb = meta_p.tile([P, TPC], mybir.dt.float32)

        x_lo = x_d[0:HALF, :]
        x_hi = x_d[HALF:NPAD, :]

        for ci in range(NCHUNKS):
            t0 = ci * CHUNK_TILES
            t1 = min(t0 + CHUNK_TILES, TPC)
            nt = t1 - t0
            ccol0 = int(chunk_col0[ci])
            ncols = nt + int(gather_nblk[ci].sum())
            s_sb = s_p.tile([P, (CHUNK_TILES + nlo_max + nhi_max) * P], mybir.dt.bfloat16, tag="sblk")
            nc.sync.dma_start(
                s_sb[:, 0 : ncols * P], s_d[:, ccol0 * P : (ccol0 + ncols) * P]
            )
            gat_self = gat_p.tile([P, CHUNK_TILES, P], mybir.dt.bfloat16, tag="gself")
            gat_lo = gat_p.tile([P, nlo_max, P], mybir.dt.bfloat16, tag="glo")
            gat_hi = gat_p.tile([P, nhi_max, P], mybir.dt.bfloat16, tag="ghi")
            gat_half = [gat_lo, gat_hi]

            # self blocks: sequential HWDGE loads of this core's own xs rows
            for t in range(t0, t1):
                nc.sync.dma_start(gat_self[:, t - t0, :], xself_d[t])

            # gathers: separate dst tiles per half -> no WAW between them, so
            # up to 4 gathers (2 chunks x 2 halves) run on 4 Q7 pairs at once.
            # Alternate half order per chunk so queue round-robin spreads the
            # (larger) lo and (smaller) hi calls evenly.
            halves = ((0, x_lo), (1, x_hi)) if ci % 2 == 0 else ((1, x_hi), (0, x_lo))
            for h, src_ap in halves:
                nblk_g = int(gather_nblk[ci, h])
                if nblk_g == 0:
                    continue
                # split into two sub-gathers on different queues; rotate the
                # queue offset per chunk so lo/hi sizes balance across queues
                nb1 = (nblk_g + 1) // 2
                for b0, b1 in ((0, nb1), (nb1, nblk_g)):
                    if b1 <= b0:
                        continue
                    nidx = (b1 - b0) * P
                    ic0 = (int(gather_gcol0[ci, h]) + b0) * 8
                    nc.gpsimd.dma_gather(
                        out_ap=gat_half[h][:, b0:b1, :],
                        in_ap=src_ap,
                        idxs_ap=idx_sb[:, ic0 : ic0 + nidx // 16],
                        num_idxs=nidx,
                        num_idxs_reg=nidx,
                        elem_size=C,
                        single_packet=False,
                        queue_num=(qctr + ci) % NQ,
                    )
                    qctr += 1

            for t in range(t0, t1):
                # (S column in s_sb, source tile, source column) per block
                blocks = [(int(col_of_self[t]) - ccol0, gat_self, t - t0)]
                for h in range(2):
                    loc0 = int(blk_gcol[t, h]) - int(gather_gcol0[ci, h])
                    for j in range(int(nb[t, h])):
                        blocks.append(
                            (int(blk_col[t, h]) + j - ccol0, gat_half[h], loc0 + j)
                        )

                pt = ps_p.tile([P, C], mybir.dt.float32, tag="ps")
                for j, (scol, gtile, gcol) in enumerate(blocks):
                    nc.tensor.matmul(
                        pt[:],
                        lhsT=gtile[:, gcol, :],
                        rhs=s_sb[:, scol * P : (scol + 1) * P],
                        start=(j == 0),
                        stop=(j == len(blocks) - 1),
                    )

                nc.scalar.copy(agg_all[:, t * C : (t + 1) * C], pt[:])

        # ---- pass 1 (floats into phase A: PE + ACT only, no DVE) ----
        sq_p2 = s_p  # reuse S pool space for square scratch
        for t in range(TPC):
            po = ps_p.tile([P, CW], mybir.dt.float32, tag="ps")
            nc.tensor.matmul(
                po[:], lhsT=agg_all[:, t * C : (t + 1) * C], rhs=wnT_sb[:],
                start=True, stop=not b_nonzero,
            )
            if b_nonzero:
                # bias' = sqrt(deg) * b (per-dst row scale folded into lhsT)
                nc.tensor.matmul(
                    po[:], lhsT=sd_sb[:, t * P : (t + 1) * P], rhs=b_sb[:],
                    start=False, stop=True,
                )
            sqj = small_p.tile([P, P], mybir.dt.bfloat16, tag="sqj")
            nc.scalar.activation(
                out=sqj[:], in_=po[:, 0:C],
                func=mybir.ActivationFunctionType.Square,
                accum_out=ssq_sb[:, t : t + 1],
            )
            nc.scalar.copy(mu_sb[:, t : t + 1], po[:, C : C + 1])

        # ---- batched norm: all [P,1]-per-tile scalars as [P,TPC] ops ----
        mu_ap = mu_sb[:]
        msq = small_p.tile([P, TPC], mybir.dt.float32, tag="msq")
        nc.vector.tensor_tensor(out=msq[:], in0=mu_ap, in1=mu_ap, op=mybir.AluOpType.mult)
        var = small_p.tile([P, TPC], mybir.dt.float32, tag="var")
        nc.vector.tensor_scalar(
            out=var[:], in0=ssq_sb[:], scalar1=1.0 / C, scalar2=None,
            op0=mybir.AluOpType.mult,
        )
        var2 = small_p.tile([P, TPC], mybir.dt.float32, tag="var2")
        nc.vector.tensor_tensor(out=var2[:], in0=var[:], in1=msq[:], op=mybir.AluOpType.subtract)
        var3 = small_p.tile([P, TPC], mybir.dt.float32, tag="var3")
        nc.vector.tensor_tensor(out=var3[:], in0=var2[:], in1=epsdeg_sb[:], op=mybir.AluOpType.add)
        std = small_p.tile([P, TPC], mybir.dt.float32, tag="std")
        nc.scalar.activation(
            out=std[:], in_=var3[:], func=mybir.ActivationFunctionType.Sqrt,
        )
        rstd = small_p.tile([P, TPC], mybir.dt.float32, tag="rstd")
        nc.vector.reciprocal(out=rstd[:], in_=std[:])
        nmr0 = small_p.tile([P, TPC], mybir.dt.float32, tag="nmr0")
        nc.vector.tensor_tensor(out=nmr0[:], in0=mu_ap, in1=rstd[:], op=mybir.AluOpType.mult)
        nmr = small_p.tile([P, TPC], mybir.dt.float32, tag="nmr")
        nc.vector.tensor_scalar(
            out=nmr[:], in0=nmr0[:], scalar1=-1.0, scalar2=None,
            op0=mybir.AluOpType.mult,
        )

        # ---- tail pass 2: recompute po on the idle PE, fused norm+LeakyReLU ----
        # wnT2 is written only after the batched norm, so pass-2 matmuls cannot
        # be scheduled early (their PSUM tiles would pin slots until Prelu).
        zcol = small_p.tile([P, 1], mybir.dt.float32, tag="zcol")
        nc.vector.tensor_scalar(
            out=zcol[:], in0=rstd[:, 0:1], scalar1=0.0, scalar2=None,
            op0=mybir.AluOpType.mult,
        )
        wnT2_sb = meta_p.tile([C, CW], mybir.dt.float32)
        nc.scalar.activation(
            out=wnT2_sb[:], in_=wnT_sb[:],
            func=mybir.ActivationFunctionType.Identity, bias=zcol[0:C, 0:1], scale=1.0,
        )
        for t in range(TPC):
            po2 = ps_p.tile([P, CW], mybir.dt.float32, tag="ps")
            nc.tensor.matmul(
                po2[:], lhsT=agg_all[:, t * C : (t + 1) * C], rhs=wnT2_sb[:],
                start=True, stop=not b_nonzero,
            )
            if b_nonzero:
                nc.tensor.matmul(
                    po2[:], lhsT=sd_sb[:, t * P : (t + 1) * P], rhs=b_sb[:],
                    start=False, stop=True,
                )
            final = out_p.tile([P, P], mybir.dt.float32, tag="final")
            # fused normalize + LeakyReLU: Prelu(po*rstd - mu*rstd, alpha=0.2)
            nc.scalar.activation(
                out=final[:], in_=po2[:, 0:C],
                func=mybir.ActivationFunctionType.Prelu,
                bias=nmr[:, t : t + 1], scale=rstd[:, t : t + 1], alpha=0.2,
            )
            nc.sync.dma_start(out_d[t * P : (t + 1) * P, :], final[:])

    nc.compile()
    return nc


def _make_in_maps(xs_pad, XSELF, IDX, SBLK, SQRTDEG, EPSDEG, WnT_ext, b_ext):
    return [
        {
            "x": xs_pad,
            "xself": np.ascontiguousarray(XSELF[i]),
            "idx": np.ascontiguousarray(IDX[i]),
            "s": np.ascontiguousarray(SBLK[i].reshape(P, -1)),
            "sqrtdeg": np.ascontiguousarray(SQRTDEG[i]),
            "epsdeg": np.ascontiguousarray(EPSDEG[i]),
            "wnT": WnT_ext,
            "b": b_ext,
        }
        for i in range(NCORES)
    ]


def kernel(x, edge_index, W, b, u):
    pre = _preprocess(x, edge_index, W, b, u)
    nc = _build(pre[-1])
    in_maps = _make_in_maps(*pre[:-1])

    # The axon terminal can be transiently unavailable right after a prior
    # process's teardown; retry with backoff.
    import time

    last_err = None
    for attempt in range(6):
        try:
            res = run_bass_kernel_spmd(nc, in_maps, list(range(NCORES)))
            break
        except Exception as e:  # noqa: BLE001
            last_err = e
            time.sleep(45)
    else:
        raise last_err
    shards = [np.asarray(res.results[i]["out"]) for i in range(NCORES)]
    out = np.concatenate(shards, axis=0)[:N]
    return out.astype(np.float32)


# revision 19
# speedup vs baseline: 1.2594x; 1.0596x over previous
"""Trainium2 Bass kernel for GCNBlock (spectral-norm linear + GCN aggregation +
InstanceNorm + LeakyReLU) distributed across 8 NeuronCores.

Strategy (per the dst-sharding hint):
  - out = A @ (x @ WnT) = (A @ x) @ WnT, where A is the symmetric-normalized
    adjacency (with self loops).  We aggregate raw x rows first, then apply the
    128x128 weight per dst tile -- no h materialization.
  - Normalization factorization: host pre-scales rows xs = dinv[src] * x.  The
    remaining per-dst scale dinv[dst] is a positive per-row factor, which
    cancels inside InstanceNorm's (z - mu)/sigma EXCEPT through the bias and
    eps; both are fixed exactly by using bias' = sqrt(deg)*b (folded into the
    bias matmul's lhsT) and eps' = deg*eps (per-row eps).  This makes every
    scatter matrix S a PURE one-hot (no coef multiply), and all self-loop
    blocks share a single constant identity S.
  - dst nodes sharded across 8 cores (6272 nodes = 49 tiles of 128 per core).
    Edges partitioned by dst on the host, sorted into per-(tile, src-half)
    groups padded to 128-edge blocks (src halves because dma_gather indices
    are int16).  Pad slots carry dstloc=-1 so their S row is all-zero.
  - Per block: bulk-gather 128 src rows of xs (bf16) via dma_gather and
    accumulate aggT[cin, dst] += Xsrc.T @ S on the PE in PSUM.  The one-hot
    scatter matrices S are PRECOMPUTED ON THE HOST (exact 0/1 in bf16) and
    streamed per chunk via a single line-rate HWDGE DMA ([e, col, dst]-major
    layout -> one contiguous descriptor per partition).  This keeps the hot
    loop entirely free of DVE work: the DVE fully blocks against SWDGE
    descriptor generation (exclusive shared SBUF port pair with GpSimd), so
    any in-loop DVE op would serialize against the gathers.
  - Self-loop rows are NOT gathered: each core's own xs slice arrives as a
    separate sequential input (xself) DMA'd per tile into a dedicated "self"
    block using the shared identity S.
  - dma_gather descriptor generation (Q7 SWDGE) is the main bottleneck at
    ~8-10ns/index per Q7 core pair.  num_swdge_queues=4 + round-robin
    queue_num spreads desc-gen over all 4 Q7 core pairs concurrently.
  - To keep the hot loop free of po-dependent ops (which would convoy the
    in-order DVE/ACT queues), the main stream does ONLY gathers + S-builds +
    aggregation matmuls + one PSUM->SBUF copy per tile into a persistent
    agg_all buffer.  The weight matmul (with a 129th column computing the row
    mean for free) runs in a tail phase and is executed TWICE on the
    otherwise-idle PE: once to collect mean (ACT copy) and sum(po^2) (ACT
    Square accum_out), and -- after a handful of BATCHED [P,49] DVE ops
    compute rstd and -mu*rstd -- a second time feeding one fused ACT
    Prelu(po*rstd - mu*rstd, alpha=0.2) per tile straight from PSUM, then the
    output DMA.
"""

import numpy as np
import ml_dtypes
from contextlib import ExitStack

import concourse.tile as tile
from concourse import bacc, mybir
from concourse.bass_utils import run_bass_kernel_spmd

_N0 = None

# Problem constants (hardcoded per spec)
N, E, C = 50000, 800000, 128
P = 128
NCORES = 8
TPC = 49                # dst tiles per core
NPC = TPC * P           # 6272 dst nodes per core
NPAD = NCORES * NPC     # 50176 padded node count
HALF = 32768            # int16 index split point
CHUNK_TILES = 3
NCHUNKS = -(-TPC // CHUNK_TILES)  # 17 (last chunk ragged)
ACT_FRAC = 5            # every 5th S-build goes to the Scalar engine
NQ = 4                  # SWDGE queues (Q7 core pairs) used for gathers
EPS_IN = 1e-5
CW = C + 1              # weight matmul width (extra column = row mean)


def _preprocess(x, edge_index, W, b, u):
    """Host-side prep: spectral norm, edge partitioning, metadata layout."""
    x = np.asarray(x, dtype=np.float32)
    ei = np.asarray(edge_index)
    W = np.asarray(W, dtype=np.float32)
    b = np.asarray(b, dtype=np.float32)
    u = np.asarray(u, dtype=np.float32)

    # --- spectral norm (one power iteration), matches reference ---
    eps = np.float32(1e-12)
    v = (W.T @ u).astype(np.float32)
    v = v / (np.float32(np.linalg.norm(v)) + eps)
    Wv = (W @ v).astype(np.float32)
    u2 = Wv / (np.float32(np.linalg.norm(Wv)) + eps)
    sigma = np.float32(u2 @ Wv)
    WnT = np.ascontiguousarray((W / sigma).T, dtype=np.float32)  # [cin, cout]
    # extended weight: col C = row-mean column (mu comes out of the matmul)
    WnT_ext = np.concatenate([WnT, WnT.mean(axis=1, keepdims=True)], axis=1)
    WnT_ext = np.ascontiguousarray(WnT_ext, dtype=np.float32)
    b_ext = np.concatenate([b, [b.mean()]]).reshape(1, CW).astype(np.float32)
    b_nonzero = bool(np.any(b))

    src = ei[0].astype(np.int64)
    dst = ei[1].astype(np.int64)

    # --- degrees; xs = dinv * x (row-scaled source features) ---
    deg = (np.bincount(dst, minlength=N) + 1).astype(np.float32)
    dinv = (1.0 / np.sqrt(deg)).astype(np.float32)
    deg_pad = np.ones(NPAD, dtype=np.float32)
    deg_pad[:N] = deg
    sqrtdeg_pad = np.sqrt(deg_pad)

    # --- group real edges by (core, tile, src-half) ---
    core = dst // NPC
    tile_g = (dst % NPC) // P
    dstloc = (dst % P).astype(np.float32)
    half = (src >= HALF).astype(np.int64)
    key = ((core * TPC + tile_g) * 2 + half).astype(np.int64)
    NG = NCORES * TPC * 2
    order = np.argsort(key, kind="stable")
    counts = np.bincount(key, minlength=NG)
    starts = np.zeros(NG + 1, dtype=np.int64)
    np.cumsum(counts, out=starts[1:])
    rank = np.arange(len(key), dtype=np.int64) - starts[key[order]]

    cnt3 = counts.reshape(NCORES, TPC, 2)
    nb = np.ceil(cnt3.max(axis=0) / P).astype(np.int64)  # [TPC, 2] gather blocks

    # Column layout per chunk: [self cols t0..t1) | lo blocks | hi blocks].
    # Gather-only column index (gcol) is separate and skips self columns.
    col_of_self = np.zeros(TPC, dtype=np.int64)
    blk_col = np.zeros((TPC, 2), dtype=np.int64)   # global gat column of block run
    blk_gcol = np.zeros((TPC, 2), dtype=np.int64)  # global gather column of run
    gather_gcol0 = np.zeros((NCHUNKS, 2), dtype=np.int64)
    gather_nblk = np.zeros((NCHUNKS, 2), dtype=np.int64)
    chunk_col0 = np.zeros(NCHUNKS, dtype=np.int64)
    cpos = 0
    gpos = 0
    for ci in range(NCHUNKS):
        t0 = ci * CHUNK_TILES
        t1 = min(t0 + CHUNK_TILES, TPC)
        chunk_col0[ci] = cpos
        for t in range(t0, t1):
            col_of_self[t] = cpos
            cpos += 1
        for h in range(2):
            gather_gcol0[ci, h] = gpos
            for t in range(t0, t1):
                blk_col[t, h] = cpos
                blk_gcol[t, h] = gpos
                cpos += nb[t, h]
                gpos += nb[t, h]
            gather_nblk[ci, h] = gpos - gather_gcol0[ci, h]
    totcol = cpos
    totg = gpos

    # host-precomputed one-hot scatter blocks S[e, col, dst] (exact 0/1 bf16);
    # [e, col, dst]-major so a chunk's S loads as one descriptor per partition
    SBLK = np.zeros((NCORES, P, totcol, P), dtype=ml_dtypes.float8_e4m3)
    IDXALL = np.zeros((NCORES, totg * P), dtype=np.int16)

    o_core = core[order]
    o_tile = tile_g[order]
    o_half = half[order]
    o_col = blk_col[o_tile, o_half] + rank // P
    o_gcol = blk_gcol[o_tile, o_half] + rank // P
    o_slot = rank % P

    SBLK[o_core, o_slot, o_col, dstloc[order].astype(np.int64)] = 1.0
    arangeP_i = np.arange(P)
    for t in range(TPC):
        SBLK[:, arangeP_i, col_of_self[t], arangeP_i] = 1.0
    IDXALL[o_core, o_gcol * P + o_slot] = (src[order] - o_half * HALF).astype(np.int16)

    # idx SBUF layout: pos k -> [k % 16, k // 16], replicated 8x over partitions
    IDX = np.tile(IDXALL.reshape(NCORES, -1, 16).transpose(0, 2, 1), (1, 8, 1))
    n0 = int(gather_nblk[0].sum())  # chunk-0 gather blocks (loaded first)

    xs_pad = np.zeros((NPAD, C), dtype=ml_dtypes.bfloat16)
    xs_pad[:N] = (dinv[:, None] * x).astype(ml_dtypes.bfloat16)
    XSELF = xs_pad.reshape(NCORES, TPC, P, C)

    SQRTDEG = sqrtdeg_pad.reshape(NCORES, 1, NPC).astype(np.float32)
    EPSDEG = (EPS_IN * deg_pad).reshape(NCORES, TPC, P).transpose(0, 2, 1)
    EPSDEG = np.ascontiguousarray(EPSDEG, dtype=np.float32)  # [NCORES, P, TPC]

    meta = dict(
        nb=nb,
        col_of_self=col_of_self,
        blk_col=blk_col,
        blk_gcol=blk_gcol,
        chunk_col0=chunk_col0,
        gather_gcol0=gather_gcol0,
        gather_nblk=gather_nblk,
        totcol=totcol,
        totg=totg,
        n0=n0,
    )
    meta['b_nonzero'] = b_nonzero
    global _N0
    _N0 = n0
    return xs_pad, XSELF, IDX, SBLK, SQRTDEG, EPSDEG, WnT_ext, b_ext, meta


def _build(meta):
    """Build the SPMD Bass graph (shared across all 8 cores)."""
    nb = meta["nb"]
    b_nonzero = meta["b_nonzero"]
    col_of_self = meta["col_of_self"]
    blk_col = meta["blk_col"]
    blk_gcol = meta["blk_gcol"]
    chunk_col0 = meta["chunk_col0"]
    gather_gcol0 = meta["gather_gcol0"]
    gather_nblk = meta["gather_nblk"]
    totcol = meta["totcol"]
    totg = meta["totg"]

    nc = bacc.Bacc(
        "TRN2", target_bir_lowering=False, debug=False, num_swdge_queues=NQ
    )

    x_d = nc.dram_tensor("x", [NPAD, C], mybir.dt.bfloat16, kind="ExternalInput")
    xself_d = nc.dram_tensor("xself", [TPC, P, C], mybir.dt.bfloat16, kind="ExternalInput")
    n0 = meta["n0"]
    idx0_d = nc.dram_tensor("idx0", [P, n0 * 8], mybir.dt.int16, kind="ExternalInput")
    idx_d = nc.dram_tensor("idx", [P, totg * 8], mybir.dt.int16, kind="ExternalInput")
    s_d = nc.dram_tensor("s", [P, totcol * P], mybir.dt.float8e4, kind="ExternalInput")
    sd_d = nc.dram_tensor("sqrtdeg", [1, NPC], mybir.dt.float32, kind="ExternalInput")
    epsdeg_d = nc.dram_tensor("epsdeg", [P, TPC], mybir.dt.float32, kind="ExternalInput")
    wnT_d = nc.dram_tensor("wnT", [C, CW], mybir.dt.float32, kind="ExternalInput")
    b_d = nc.dram_tensor("b", [1, CW], mybir.dt.float32, kind="ExternalInput")
    out_d = nc.dram_tensor("out", [NPC, C], mybir.dt.float32, kind="ExternalOutput")

    # max gather blocks per chunk for each half (separate tiles per half)
    nlo_max = max(int(gather_nblk[ci, 0]) for ci in range(NCHUNKS))
    nhi_max = max(int(gather_nblk[ci, 1]) for ci in range(NCHUNKS))

    qctr = 0  # gather round-robin queue counter

    with tile.TileContext(nc) as tc, ExitStack() as ctx:
        meta_p = ctx.enter_context(tc.tile_pool(name="meta", bufs=1))
        gat_p = ctx.enter_context(tc.tile_pool(name="gat", bufs=6))
        s_p = ctx.enter_context(tc.tile_pool(name="s", bufs=6))
        agg_p = ctx.enter_context(tc.tile_pool(name="agg", bufs=4))
        out_p = ctx.enter_context(tc.tile_pool(name="out", bufs=6))
        small_p = ctx.enter_context(tc.tile_pool(name="small", bufs=1))
        ps_p = ctx.enter_context(tc.tile_pool(name="ps", bufs=8, space="PSUM"))

        idxz = meta_p.tile([P, 8], mybir.dt.int16)
        nc.vector.memset(idxz[:], 0)
        warm = meta_p.tile([P, 1, P], mybir.dt.bfloat16)
        nc.gpsimd.dma_gather(
            out_ap=warm[:], in_ap=x_d[0:HALF, :], idxs_ap=idxz[:, 0:8],
            num_idxs=P, num_idxs_reg=P, elem_size=C,
            single_packet=False, queue_num=0,
        )
        idx0_sb = meta_p.tile([P, n0 * 8], mybir.dt.int16)
        nc.sync.dma_start(idx0_sb[:], idx0_d[:])
        idx_sb = meta_p.tile([P, totg * 8], mybir.dt.int16)
        nc.sync.dma_start(idx_sb[:], idx_d[:])
        sd_sb = meta_p.tile([1, NPC], mybir.dt.float32)
        nc.sync.dma_start(sd_sb[:], sd_d[:])
        epsdeg_sb = meta_p.tile([P, TPC], mybir.dt.float32)
        nc.sync.dma_start(epsdeg_sb[:], epsdeg_d[:])
        wnT_sb = meta_p.tile([C, CW], mybir.dt.float32)
        nc.sync.dma_start(wnT_sb[:], wnT_d[:])
        b_sb = meta_p.tile([1, CW], mybir.dt.float32)
        nc.sync.dma_start(b_sb[:], b_d[:])

        # persistent aggregation output + norm-stat staging for the tail
        agg_all = meta_p.tile([P, TPC * C], mybir.dt.float32)
        mu_sb = meta_p.tile([P, TPC], mybir.dt.float32)
        ssq_sb = meta_p.tile([P, TPC], mybir.dt.float32)

        x_lo = x_d[0:HALF, :]
        x_hi = x_d[HALF:NPAD, :]

        for ci in range(NCHUNKS):
            t0 = ci * CHUNK_TILES
            t1 = min(t0 + CHUNK_TILES, TPC)
            nt = t1 - t0
            ccol0 = int(chunk_col0[ci])
            ncols = nt + int(gather_nblk[ci].sum())
            s_sb = s_p.tile([P, (CHUNK_TILES + nlo_max + nhi_max) * P], mybir.dt.float8e4, tag="sblk")
            nc.sync.dma_start(
                s_sb[:, 0 : ncols * P], s_d[:, ccol0 * P : (ccol0 + ncols) * P]
            )
            gat_self = gat_p.tile([P, CHUNK_TILES, P], mybir.dt.bfloat16, tag="gself")
            gat_lo = gat_p.tile([P, nlo_max, P], mybir.dt.bfloat16, tag="glo")
            gat_hi = gat_p.tile([P, nhi_max, P], mybir.dt.bfloat16, tag="ghi")
            gat_half = [gat_lo, gat_hi]

            # self blocks: sequential HWDGE loads of this core's own xs rows
            for t in range(t0, t1):
                nc.sync.dma_start(gat_self[:, t - t0, :], xself_d[t])

            # gathers: separate dst tiles per half -> no WAW between them, so
            # up to 4 gathers (2 chunks x 2 halves) run on 4 Q7 pairs at once.
            # Alternate half order per chunk so queue round-robin spreads the
            # (larger) lo and (smaller) hi calls evenly.
            halves = ((0, x_lo), (1, x_hi)) if ci % 2 == 0 else ((1, x_hi), (0, x_lo))
            for h, src_ap in halves:
                nblk_g = int(gather_nblk[ci, h])
                if nblk_g == 0:
                    continue
                # split into two sub-gathers on different queues; rotate the
                # queue offset per chunk so lo/hi sizes balance across queues
                nb1 = (nblk_g + 1) // 2
                for b0, b1 in ((0, nb1), (nb1, nblk_g)):
                    if b1 <= b0:
                        continue
                    nidx = (b1 - b0) * P
                    ic0 = (int(gather_gcol0[ci, h]) + b0) * 8
                    idx_src = idx0_sb if ci == 0 else idx_sb
                    nc.gpsimd.dma_gather(
                        out_ap=gat_half[h][:, b0:b1, :],
                        in_ap=src_ap,
                        idxs_ap=idx_src[:, ic0 : ic0 + nidx // 16],
                        num_idxs=nidx,
                        num_idxs_reg=nidx,
                        elem_size=C,
                        single_packet=False,
                        queue_num=(qctr + ci) % NQ,
                    )
                    qctr += 1

            for t in range(t0, t1):
                # (S column in s_sb, source tile, source column) per block
                blocks = [(int(col_of_self[t]) - ccol0, gat_self, t - t0)]
                for h in range(2):
                    loc0 = int(blk_gcol[t, h]) - int(gather_gcol0[ci, h])
                    for j in range(int(nb[t, h])):
                        blocks.append(
                            (int(blk_col[t, h]) + j - ccol0, gat_half[h], loc0 + j)
                        )

                pt = ps_p.tile([P, C], mybir.dt.float32, tag="ps")
                for j, (scol, gtile, gcol) in enumerate(blocks):
                    nc.tensor.matmul(
                        pt[:],
                        lhsT=gtile[:, gcol, :],
                        rhs=s_sb[:, scol * P : (scol + 1) * P],
                        start=(j == 0),
                        stop=(j == len(blocks) - 1),
                    )

                nc.scalar.copy(agg_all[:, t * C : (t + 1) * C], pt[:])

        # ---- pass 1 (floats into phase A: PE + ACT only, no DVE) ----
        sq_p2 = s_p  # reuse S pool space for square scratch
        for t in range(TPC):
            po = ps_p.tile([P, CW], mybir.dt.float32, tag="ps")
            nc.tensor.matmul(
                po[:], lhsT=agg_all[:, t * C : (t + 1) * C], rhs=wnT_sb[:],
                start=True, stop=not b_nonzero,
            )
            if b_nonzero:
                # bias' = sqrt(deg) * b (per-dst row scale folded into lhsT)
                nc.tensor.matmul(
                    po[:], lhsT=sd_sb[:, t * P : (t + 1) * P], rhs=b_sb[:],
                    start=False, stop=True,
                )
            sqj = small_p.tile([P, P], mybir.dt.bfloat16, tag="sqj")
            nc.scalar.activation(
                out=sqj[:], in_=po[:, 0:C],
                func=mybir.ActivationFunctionType.Square,
                accum_out=ssq_sb[:, t : t + 1],
            )
            nc.scalar.copy(mu_sb[:, t : t + 1], po[:, C : C + 1])

        # ---- batched norm: all [P,1]-per-tile scalars as [P,TPC] ops ----
        mu_ap = mu_sb[:]
        msq = small_p.tile([P, TPC], mybir.dt.float32, tag="msq")
        nc.vector.tensor_tensor(out=msq[:], in0=mu_ap, in1=mu_ap, op=mybir.AluOpType.mult)
        var = small_p.tile([P, TPC], mybir.dt.float32, tag="var")
        nc.vector.tensor_scalar(
            out=var[:], in0=ssq_sb[:], scalar1=1.0 / C, scalar2=None,
            op0=mybir.AluOpType.mult,
        )
        var2 = small_p.tile([P, TPC], mybir.dt.float32, tag="var2")
        nc.vector.tensor_tensor(out=var2[:], in0=var[:], in1=msq[:], op=mybir.AluOpType.subtract)
        var3 = small_p.tile([P, TPC], mybir.dt.float32, tag="var3")
        nc.vector.tensor_tensor(out=var3[:], in0=var2[:], in1=epsdeg_sb[:], op=mybir.AluOpType.add)
        std = small_p.tile([P, TPC], mybir.dt.float32, tag="std")
        nc.scalar.activation(
            out=std[:], in_=var3[:], func=mybir.ActivationFunctionType.Sqrt,
        )
        rstd = small_p.tile([P, TPC], mybir.dt.float32, tag="rstd")
        nc.vector.reciprocal(out=rstd[:], in_=std[:])
        nmr0 = small_p.tile([P, TPC], mybir.dt.float32, tag="nmr0")
        nc.vector.tensor_tensor(out=nmr0[:], in0=mu_ap, in1=rstd[:], op=mybir.AluOpType.mult)
        nmr = small_p.tile([P, TPC], mybir.dt.float32, tag="nmr")
        nc.vector.tensor_scalar(
            out=nmr[:], in0=nmr0[:], scalar1=-1.0, scalar2=None,
            op0=mybir.AluOpType.mult,
        )

        # ---- tail pass 2: recompute po on the idle PE, fused norm+LeakyReLU ----
        # wnT2 is written only after the batched norm, so pass-2 matmuls cannot
        # be scheduled early (their PSUM tiles would pin slots until Prelu).
        zcol = small_p.tile([P, 1], mybir.dt.float32, tag="zcol")
        nc.vector.tensor_scalar(
            out=zcol[:], in0=rstd[:, 0:1], scalar1=0.0, scalar2=None,
            op0=mybir.AluOpType.mult,
        )
        wnT2_sb = meta_p.tile([C, CW], mybir.dt.float32)
        nc.scalar.activation(
            out=wnT2_sb[:], in_=wnT_sb[:],
            func=mybir.ActivationFunctionType.Identity, bias=zcol[0:C, 0:1], scale=1.0,
        )
        for t in range(TPC):
            po2 = ps_p.tile([P, CW], mybir.dt.float32, tag="ps")
            nc.tensor.matmul(
                po2[:], lhsT=agg_all[:, t * C : (t + 1) * C], rhs=wnT2_sb[:],
                start=True, stop=not b_nonzero,
            )
            if b_nonzero:
                nc.tensor.matmul(
                    po2[:], lhsT=sd_sb[:, t * P : (t + 1) * P], rhs=b_sb[:],
                    start=False, stop=True,
                )
            final = out_p.tile([P, P], mybir.dt.float32, tag="final")
            # fused normalize + LeakyReLU: Prelu(po*rstd - mu*rstd, alpha=0.2)
            nc.scalar.activation(
                out=final[:], in_=po2[:, 0:C],
                func=mybir.ActivationFunctionType.Prelu,
                bias=nmr[:, t : t + 1], scale=rstd[:, t : t + 1], alpha=0.2,
            )
            nc.sync.dma_start(out_d[t * P : (t + 1) * P, :], final[:])

    nc.compile()
    return nc


_N0 = None


def _make_in_maps(xs_pad, XSELF, IDX, SBLK, SQRTDEG, EPSDEG, WnT_ext, b_ext):
    return [
        {
            "x": xs_pad,
            "xself": np.ascontiguousarray(XSELF[i]),
            "idx": np.ascontiguousarray(IDX[i]),
            "idx0": np.ascontiguousarray(IDX[i][:, : _N0 * 8]),
            "s": np.ascontiguousarray(SBLK[i].reshape(P, -1)),
            "sqrtdeg": np.ascontiguousarray(SQRTDEG[i]),
            "epsdeg": np.ascontiguousarray(EPSDEG[i]),
            "wnT": WnT_ext,
            "b": b_ext,
        }
        for i in range(NCORES)
    ]


def kernel(x, edge_index, W, b, u):
    pre = _preprocess(x, edge_index, W, b, u)
    nc = _build(pre[-1])
    in_maps = _make_in_maps(*pre[:-1])

    # The axon terminal can be transiently unavailable right after a prior
    # process's teardown; retry with backoff.
    import time

    last_err = None
    for attempt in range(6):
        try:
            res = run_bass_kernel_spmd(nc, in_maps, list(range(NCORES)))
            break
        except Exception as e:  # noqa: BLE001
            last_err = e
            time.sleep(45)
    else:
        raise last_err
    shards = [np.asarray(res.results[i]["out"]) for i in range(NCORES)]
    out = np.concatenate(shards, axis=0)[:N]
    return out.astype(np.float32)


# revision 20
# speedup vs baseline: 1.2789x; 1.0155x over previous
"""Trainium2 Bass kernel for GCNBlock (spectral-norm linear + GCN aggregation +
InstanceNorm + LeakyReLU) distributed across 8 NeuronCores.

Strategy (per the dst-sharding hint):
  - out = A @ (x @ WnT) = (A @ x) @ WnT, where A is the symmetric-normalized
    adjacency (with self loops).  We aggregate raw x rows first, then apply the
    128x128 weight per dst tile -- no h materialization.
  - Normalization factorization: host pre-scales rows xs = dinv[src] * x.  The
    remaining per-dst scale dinv[dst] is a positive per-row factor, which
    cancels inside InstanceNorm's (z - mu)/sigma EXCEPT through the bias and
    eps; both are fixed exactly by using bias' = sqrt(deg)*b (folded into the
    bias matmul's lhsT) and eps' = deg*eps (per-row eps).  This makes every
    scatter matrix S a PURE one-hot (no coef multiply), and all self-loop
    blocks share a single constant identity S.
  - dst nodes sharded across 8 cores (6272 nodes = 49 tiles of 128 per core).
    Edges partitioned by dst on the host, sorted into per-(tile, src-half)
    groups padded to 128-edge blocks (src halves because dma_gather indices
    are int16).  Pad slots carry dstloc=-1 so their S row is all-zero.
  - Per block: bulk-gather 128 src rows of xs (bf16) via dma_gather and
    accumulate aggT[cin, dst] += Xsrc.T @ S on the PE in PSUM.  The one-hot
    scatter matrices S are PRECOMPUTED ON THE HOST (exact 0/1 in bf16) and
    streamed per chunk via a single line-rate HWDGE DMA ([e, col, dst]-major
    layout -> one contiguous descriptor per partition).  This keeps the hot
    loop entirely free of DVE work: the DVE fully blocks against SWDGE
    descriptor generation (exclusive shared SBUF port pair with GpSimd), so
    any in-loop DVE op would serialize against the gathers.
  - Self-loop rows are NOT gathered: each core's own xs slice arrives as a
    separate sequential input (xself) DMA'd per tile into a dedicated "self"
    block using the shared identity S.
  - dma_gather descriptor generation (Q7 SWDGE) is the main bottleneck at
    ~8-10ns/index per Q7 core pair.  num_swdge_queues=4 + round-robin
    queue_num spreads desc-gen over all 4 Q7 core pairs concurrently.
  - To keep the hot loop free of po-dependent ops (which would convoy the
    in-order DVE/ACT queues), the main stream does ONLY gathers + S-builds +
    aggregation matmuls + one PSUM->SBUF copy per tile into a persistent
    agg_all buffer.  The weight matmul (with a 129th column computing the row
    mean for free) runs in a tail phase and is executed TWICE on the
    otherwise-idle PE: once to collect mean (ACT copy) and sum(po^2) (ACT
    Square accum_out), and -- after a handful of BATCHED [P,49] DVE ops
    compute rstd and -mu*rstd -- a second time feeding one fused ACT
    Prelu(po*rstd - mu*rstd, alpha=0.2) per tile straight from PSUM, then the
    output DMA.
"""

import numpy as np
import ml_dtypes
from contextlib import ExitStack

import concourse.tile as tile
from concourse import bacc, mybir
from concourse.bass_utils import run_bass_kernel_spmd

_N0 = None

# Problem constants (hardcoded per spec)
N, E, C = 50000, 800000, 128
P = 128
NCORES = 8
TPC = 49                # dst tiles per core
NPC = TPC * P           # 6272 dst nodes per core
NPAD = NCORES * NPC     # 50176 padded node count
HALF = 32768            # int16 index split point
CHUNK_TILES = 3
NCHUNKS = -(-TPC // CHUNK_TILES)  # 17 (last chunk ragged)
ACT_FRAC = 5            # every 5th S-build goes to the Scalar engine
NQ = 4                  # SWDGE queues (Q7 core pairs) used for gathers
EPS_IN = 1e-5
CW = C + 1              # weight matmul width (extra column = row mean)


def _preprocess(x, edge_index, W, b, u):
    """Host-side prep: spectral norm, edge partitioning, metadata layout."""
    x = np.asarray(x, dtype=np.float32)
    ei = np.asarray(edge_index)
    W = np.asarray(W, dtype=np.float32)
    b = np.asarray(b, dtype=np.float32)
    u = np.asarray(u, dtype=np.float32)

    # --- spectral norm (one power iteration), matches reference ---
    eps = np.float32(1e-12)
    v = (W.T @ u).astype(np.float32)
    v = v / (np.float32(np.linalg.norm(v)) + eps)
    Wv = (W @ v).astype(np.float32)
    u2 = Wv / (np.float32(np.linalg.norm(Wv)) + eps)
    sigma = np.float32(u2 @ Wv)
    WnT = np.ascontiguousarray((W / sigma).T, dtype=np.float32)  # [cin, cout]
    # extended weight: col C = row-mean column (mu comes out of the matmul)
    WnT_ext = np.concatenate([WnT, WnT.mean(axis=1, keepdims=True)], axis=1)
    WnT_ext = np.ascontiguousarray(WnT_ext, dtype=np.float32)
    b_ext = np.concatenate([b, [b.mean()]]).reshape(1, CW).astype(np.float32)
    b_nonzero = bool(np.any(b))

    src = ei[0].astype(np.int64)
    dst = ei[1].astype(np.int64)

    # --- degrees; xs = dinv * x (row-scaled source features) ---
    deg = (np.bincount(dst, minlength=N) + 1).astype(np.float32)
    dinv = (1.0 / np.sqrt(deg)).astype(np.float32)
    deg_pad = np.ones(NPAD, dtype=np.float32)
    deg_pad[:N] = deg
    sqrtdeg_pad = np.sqrt(deg_pad)

    # --- group real edges by (core, tile, src-half) ---
    core = dst // NPC
    tile_g = (dst % NPC) // P
    dstloc = (dst % P).astype(np.float32)
    half = (src >= HALF).astype(np.int64)
    key = ((core * TPC + tile_g) * 2 + half).astype(np.int64)
    NG = NCORES * TPC * 2
    order = np.argsort(key, kind="stable")
    counts = np.bincount(key, minlength=NG)
    starts = np.zeros(NG + 1, dtype=np.int64)
    np.cumsum(counts, out=starts[1:])
    rank = np.arange(len(key), dtype=np.int64) - starts[key[order]]

    cnt3 = counts.reshape(NCORES, TPC, 2)
    nb = np.ceil(cnt3.max(axis=0) / P).astype(np.int64)  # [TPC, 2] gather blocks

    # Column layout per chunk: [self cols t0..t1) | lo blocks | hi blocks].
    # Gather-only column index (gcol) is separate and skips self columns.
    col_of_self = np.zeros(TPC, dtype=np.int64)
    blk_col = np.zeros((TPC, 2), dtype=np.int64)   # global gat column of block run
    blk_gcol = np.zeros((TPC, 2), dtype=np.int64)  # global gather column of run
    gather_gcol0 = np.zeros((NCHUNKS, 2), dtype=np.int64)
    gather_nblk = np.zeros((NCHUNKS, 2), dtype=np.int64)
    chunk_col0 = np.zeros(NCHUNKS, dtype=np.int64)
    cpos = 0
    gpos = 0
    for ci in range(NCHUNKS):
        t0 = ci * CHUNK_TILES
        t1 = min(t0 + CHUNK_TILES, TPC)
        chunk_col0[ci] = cpos
        for t in range(t0, t1):
            col_of_self[t] = cpos
            cpos += 1
        for h in range(2):
            gather_gcol0[ci, h] = gpos
            for t in range(t0, t1):
                blk_col[t, h] = cpos
                blk_gcol[t, h] = gpos
                cpos += nb[t, h]
                gpos += nb[t, h]
            gather_nblk[ci, h] = gpos - gather_gcol0[ci, h]
    totcol = cpos
    totg = gpos

    # host-precomputed one-hot scatter blocks S[e, col, dst] (exact 0/1 bf16);
    # [e, col, dst]-major so a chunk's S loads as one descriptor per partition
    SBLK = np.zeros((NCORES, P, totcol, P), dtype=ml_dtypes.float8_e4m3)
    IDXALL = np.zeros((NCORES, totg * P), dtype=np.int16)

    o_core = core[order]
    o_tile = tile_g[order]
    o_half = half[order]
    o_col = blk_col[o_tile, o_half] + rank // P
    o_gcol = blk_gcol[o_tile, o_half] + rank // P
    o_slot = rank % P

    SBLK[o_core, o_slot, o_col, dstloc[order].astype(np.int64)] = 1.0
    arangeP_i = np.arange(P)
    for t in range(TPC):
        SBLK[:, arangeP_i, col_of_self[t], arangeP_i] = 1.0
    IDXALL[o_core, o_gcol * P + o_slot] = (src[order] - o_half * HALF).astype(np.int16)

    # idx SBUF layout: pos k -> [k % 16, k // 16], replicated 8x over partitions
    IDX = np.tile(IDXALL.reshape(NCORES, -1, 16).transpose(0, 2, 1), (1, 8, 1))
    n0 = int(gather_nblk[0].sum())  # chunk-0 gather blocks (loaded first)

    xs_pad = np.zeros((NPAD, C), dtype=ml_dtypes.bfloat16)
    xs_pad[:N] = (dinv[:, None] * x).astype(ml_dtypes.bfloat16)
    XSELF = xs_pad.reshape(NCORES, TPC, P, C)

    SQRTDEG = sqrtdeg_pad.reshape(NCORES, 1, NPC).astype(np.float32)
    EPSDEG = (EPS_IN * deg_pad).reshape(NCORES, TPC, P).transpose(0, 2, 1)
    EPSDEG = np.ascontiguousarray(EPSDEG, dtype=np.float32)  # [NCORES, P, TPC]

    meta = dict(
        nb=nb,
        col_of_self=col_of_self,
        blk_col=blk_col,
        blk_gcol=blk_gcol,
        chunk_col0=chunk_col0,
        gather_gcol0=gather_gcol0,
        gather_nblk=gather_nblk,
        totcol=totcol,
        totg=totg,
        n0=n0,
    )
    meta['b_nonzero'] = b_nonzero
    global _N0
    _N0 = n0
    return xs_pad, XSELF, IDX, SBLK, SQRTDEG, EPSDEG, WnT_ext, b_ext, meta


def _build(meta):
    """Build the SPMD Bass graph (shared across all 8 cores)."""
    nb = meta["nb"]
    b_nonzero = meta["b_nonzero"]
    col_of_self = meta["col_of_self"]
    blk_col = meta["blk_col"]
    blk_gcol = meta["blk_gcol"]
    chunk_col0 = meta["chunk_col0"]
    gather_gcol0 = meta["gather_gcol0"]
    gather_nblk = meta["gather_nblk"]
    totcol = meta["totcol"]
    totg = meta["totg"]

    nc = bacc.Bacc(
        "TRN2", target_bir_lowering=False, debug=False, num_swdge_queues=NQ
    )

    x_d = nc.dram_tensor("x", [NPAD, C], mybir.dt.bfloat16, kind="ExternalInput")
    xself_d = nc.dram_tensor("xself", [TPC, P, C], mybir.dt.bfloat16, kind="ExternalInput")
    n0 = meta["n0"]
    idx0_d = nc.dram_tensor("idx0", [P, n0 * 8], mybir.dt.int16, kind="ExternalInput")
    idx_d = nc.dram_tensor("idx", [P, totg * 8], mybir.dt.int16, kind="ExternalInput")
    s_d = nc.dram_tensor("s", [P, totcol * P], mybir.dt.float8e4, kind="ExternalInput")
    sd_d = nc.dram_tensor("sqrtdeg", [1, NPC], mybir.dt.float32, kind="ExternalInput")
    epsdeg_d = nc.dram_tensor("epsdeg", [P, TPC], mybir.dt.float32, kind="ExternalInput")
    wnT_d = nc.dram_tensor("wnT", [C, CW], mybir.dt.float32, kind="ExternalInput")
    b_d = nc.dram_tensor("b", [1, CW], mybir.dt.float32, kind="ExternalInput")
    out_d = nc.dram_tensor("out", [NPC, C], mybir.dt.float32, kind="ExternalOutput")

    # max gather blocks per chunk for each half (separate tiles per half)
    nlo_max = max(int(gather_nblk[ci, 0]) for ci in range(NCHUNKS))
    nhi_max = max(int(gather_nblk[ci, 1]) for ci in range(NCHUNKS))

    qctr = 0  # gather round-robin queue counter

    with tile.TileContext(nc) as tc, ExitStack() as ctx:
        meta_p = ctx.enter_context(tc.tile_pool(name="meta", bufs=1))
        gat_p = ctx.enter_context(tc.tile_pool(name="gat", bufs=6))
        s_p = ctx.enter_context(tc.tile_pool(name="s", bufs=6))
        agg_p = ctx.enter_context(tc.tile_pool(name="agg", bufs=4))
        out_p = ctx.enter_context(tc.tile_pool(name="out", bufs=6))
        small_p = ctx.enter_context(tc.tile_pool(name="small", bufs=1))
        ps_p = ctx.enter_context(tc.tile_pool(name="ps", bufs=8, space="PSUM"))

        idxz = meta_p.tile([P, 8], mybir.dt.int16)
        nc.vector.memset(idxz[:], 0)
        warm = meta_p.tile([P, 1, P], mybir.dt.bfloat16)
        nc.gpsimd.dma_gather(
            out_ap=warm[:], in_ap=x_d[0:HALF, :], idxs_ap=idxz[:, 0:8],
            num_idxs=P, num_idxs_reg=P, elem_size=C,
            single_packet=False, queue_num=0,
        )
        idx0_sb = meta_p.tile([P, n0 * 8], mybir.dt.int16)
        nc.sync.dma_start(idx0_sb[:], idx0_d[:])
        idx_sb = meta_p.tile([P, totg * 8], mybir.dt.int16)
        nc.sync.dma_start(idx_sb[:], idx_d[:])
        sd_sb = meta_p.tile([1, NPC], mybir.dt.float32)
        nc.sync.dma_start(sd_sb[:], sd_d[:])
        epsdeg_sb = meta_p.tile([P, TPC], mybir.dt.float32)
        nc.sync.dma_start(epsdeg_sb[:], epsdeg_d[:])
        wnT_sb = meta_p.tile([C, CW], mybir.dt.float32)
        nc.sync.dma_start(wnT_sb[:], wnT_d[:])
        b_sb = meta_p.tile([1, CW], mybir.dt.float32)
        nc.sync.dma_start(b_sb[:], b_d[:])

        # persistent aggregation output + norm-stat staging for the tail;
        # stats are split in two tile batches so the first batch's norm + B2
        # can overlap the last chunks' drain
        SPLIT = 32
        agg_all = meta_p.tile([P, TPC * C], mybir.dt.float32)
        mu0_sb = meta_p.tile([P, SPLIT], mybir.dt.float32)
        ssq0_sb = meta_p.tile([P, SPLIT], mybir.dt.float32)
        mu1_sb = meta_p.tile([P, TPC - SPLIT], mybir.dt.float32)
        ssq1_sb = meta_p.tile([P, TPC - SPLIT], mybir.dt.float32)

        x_lo = x_d[0:HALF, :]
        x_hi = x_d[HALF:NPAD, :]

        for ci in range(NCHUNKS):
            t0 = ci * CHUNK_TILES
            t1 = min(t0 + CHUNK_TILES, TPC)
            nt = t1 - t0
            ccol0 = int(chunk_col0[ci])
            ncols = nt + int(gather_nblk[ci].sum())
            s_sb = s_p.tile([P, (CHUNK_TILES + nlo_max + nhi_max) * P], mybir.dt.float8e4, tag="sblk")
            nc.sync.dma_start(
                s_sb[:, 0 : ncols * P], s_d[:, ccol0 * P : (ccol0 + ncols) * P]
            )
            gat_self = gat_p.tile([P, CHUNK_TILES, P], mybir.dt.bfloat16, tag="gself")
            gat_lo = gat_p.tile([P, nlo_max, P], mybir.dt.bfloat16, tag="glo")
            gat_hi = gat_p.tile([P, nhi_max, P], mybir.dt.bfloat16, tag="ghi")
            gat_half = [gat_lo, gat_hi]

            # self blocks: sequential HWDGE loads of this core's own xs rows
            for t in range(t0, t1):
                nc.sync.dma_start(gat_self[:, t - t0, :], xself_d[t])

            # gathers: separate dst tiles per half -> no WAW between them, so
            # up to 4 gathers (2 chunks x 2 halves) run on 4 Q7 pairs at once.
            # Alternate half order per chunk so queue round-robin spreads the
            # (larger) lo and (smaller) hi calls evenly.
            halves = ((0, x_lo), (1, x_hi)) if ci % 2 == 0 else ((1, x_hi), (0, x_lo))
            for h, src_ap in halves:
                nblk_g = int(gather_nblk[ci, h])
                if nblk_g == 0:
                    continue
                # split into two sub-gathers on different queues; rotate the
                # queue offset per chunk so lo/hi sizes balance across queues
                nb1 = (nblk_g + 1) // 2
                for b0, b1 in ((0, nb1), (nb1, nblk_g)):
                    if b1 <= b0:
                        continue
                    nidx = (b1 - b0) * P
                    ic0 = (int(gather_gcol0[ci, h]) + b0) * 8
                    idx_src = idx0_sb if ci == 0 else idx_sb
                    nc.gpsimd.dma_gather(
                        out_ap=gat_half[h][:, b0:b1, :],
                        in_ap=src_ap,
                        idxs_ap=idx_src[:, ic0 : ic0 + nidx // 16],
                        num_idxs=nidx,
                        num_idxs_reg=nidx,
                        elem_size=C,
                        single_packet=False,
                        queue_num=(qctr + ci) % NQ,
                    )
                    qctr += 1

            for t in range(t0, t1):
                # (S column in s_sb, source tile, source column) per block
                blocks = [(int(col_of_self[t]) - ccol0, gat_self, t - t0)]
                for h in range(2):
                    loc0 = int(blk_gcol[t, h]) - int(gather_gcol0[ci, h])
                    for j in range(int(nb[t, h])):
                        blocks.append(
                            (int(blk_col[t, h]) + j - ccol0, gat_half[h], loc0 + j)
                        )

                pt = ps_p.tile([P, C], mybir.dt.float32, tag="ps")
                for j, (scol, gtile, gcol) in enumerate(blocks):
                    nc.tensor.matmul(
                        pt[:],
                        lhsT=gtile[:, gcol, :],
                        rhs=s_sb[:, scol * P : (scol + 1) * P],
                        start=(j == 0),
                        stop=(j == len(blocks) - 1),
                    )

                nc.scalar.copy(agg_all[:, t * C : (t + 1) * C], pt[:])

        # ---- pass 1 (floats into phase A: PE + ACT only, no DVE) ----
        sq_p2 = s_p  # reuse S pool space for square scratch
        for t in range(TPC):
            po = ps_p.tile([P, CW], mybir.dt.float32, tag="ps")
            nc.tensor.matmul(
                po[:], lhsT=agg_all[:, t * C : (t + 1) * C], rhs=wnT_sb[:],
                start=True, stop=not b_nonzero,
            )
            if b_nonzero:
                # bias' = sqrt(deg) * b (per-dst row scale folded into lhsT)
                nc.tensor.matmul(
                    po[:], lhsT=sd_sb[:, t * P : (t + 1) * P], rhs=b_sb[:],
                    start=False, stop=True,
                )
            sqj = small_p.tile([P, P], mybir.dt.bfloat16, tag="sqj")
            ssq_x, mu_x, tb = (
                (ssq0_sb, mu0_sb, t) if t < SPLIT else (ssq1_sb, mu1_sb, t - SPLIT)
            )
            nc.scalar.activation(
                out=sqj[:], in_=po[:, 0:C],
                func=mybir.ActivationFunctionType.Square,
                accum_out=ssq_x[:, tb : tb + 1],
            )
            nc.scalar.copy(mu_x[:, tb : tb + 1], po[:, C : C + 1])

        # ---- per-batch: batched norm scalars, then fused Prelu + output DMA ----
        for g, (tb0, tb1, mu_x, ssq_x) in enumerate(
            ((0, SPLIT, mu0_sb, ssq0_sb), (SPLIT, TPC, mu1_sb, ssq1_sb))
        ):
            nt_b = tb1 - tb0
            mu_ap = mu_x[:]
            msq = small_p.tile([P, nt_b], mybir.dt.float32, tag=f"msq{g}")
            nc.vector.tensor_tensor(out=msq[:], in0=mu_ap, in1=mu_ap, op=mybir.AluOpType.mult)
            var = small_p.tile([P, nt_b], mybir.dt.float32, tag=f"var{g}")
            nc.vector.tensor_scalar(
                out=var[:], in0=ssq_x[:], scalar1=1.0 / C, scalar2=None,
                op0=mybir.AluOpType.mult,
            )
            var2 = small_p.tile([P, nt_b], mybir.dt.float32, tag=f"var2{g}")
            nc.vector.tensor_tensor(out=var2[:], in0=var[:], in1=msq[:], op=mybir.AluOpType.subtract)
            var3 = small_p.tile([P, nt_b], mybir.dt.float32, tag=f"var3{g}")
            nc.vector.tensor_tensor(
                out=var3[:], in0=var2[:], in1=epsdeg_sb[:, tb0:tb1], op=mybir.AluOpType.add
            )
            std = small_p.tile([P, nt_b], mybir.dt.float32, tag=f"std{g}")
            nc.scalar.activation(
                out=std[:], in_=var3[:], func=mybir.ActivationFunctionType.Sqrt,
            )
            rstd = small_p.tile([P, nt_b], mybir.dt.float32, tag=f"rstd{g}")
            nc.vector.reciprocal(out=rstd[:], in_=std[:])
            nmr0 = small_p.tile([P, nt_b], mybir.dt.float32, tag=f"nmr0{g}")
            nc.vector.tensor_tensor(out=nmr0[:], in0=mu_ap, in1=rstd[:], op=mybir.AluOpType.mult)
            nmr = small_p.tile([P, nt_b], mybir.dt.float32, tag=f"nmr{g}")
            nc.vector.tensor_scalar(
                out=nmr[:], in0=nmr0[:], scalar1=-1.0, scalar2=None,
                op0=mybir.AluOpType.mult,
            )

            # gate this batch's pass-2 matmuls on its rstd so their PSUM tiles
            # cannot pin slots before the norm scalars exist
            zcol = small_p.tile([P, 1], mybir.dt.float32, tag=f"zcol{g}")
            nc.vector.tensor_scalar(
                out=zcol[:], in0=rstd[:, 0:1], scalar1=0.0, scalar2=None,
                op0=mybir.AluOpType.mult,
            )
            wnT2_sb = meta_p.tile([C, CW], mybir.dt.float32, tag=f"wnT2{g}")
            nc.scalar.activation(
                out=wnT2_sb[:], in_=wnT_sb[:],
                func=mybir.ActivationFunctionType.Identity, bias=zcol[0:C, 0:1], scale=1.0,
            )
            for t in range(tb0, tb1):
                po2 = ps_p.tile([P, CW], mybir.dt.float32, tag="ps")
                nc.tensor.matmul(
                    po2[:], lhsT=agg_all[:, t * C : (t + 1) * C], rhs=wnT2_sb[:],
                    start=True, stop=not b_nonzero,
                )
                if b_nonzero:
                    nc.tensor.matmul(
                        po2[:], lhsT=sd_sb[:, t * P : (t + 1) * P], rhs=b_sb[:],
                        start=False, stop=True,
                    )
                final = out_p.tile([P, P], mybir.dt.float32, tag="final")
                # fused normalize + LeakyReLU: Prelu(po*rstd - mu*rstd, alpha=0.2)
                nc.scalar.activation(
                    out=final[:], in_=po2[:, 0:C],
                    func=mybir.ActivationFunctionType.Prelu,
                    bias=nmr[:, t - tb0 : t - tb0 + 1], scale=rstd[:, t - tb0 : t - tb0 + 1], alpha=0.2,
                )
                nc.sync.dma_start(out_d[t * P : (t + 1) * P, :], final[:])

    nc.compile()
    return nc


_N0 = None


def _make_in_maps(xs_pad, XSELF, IDX, SBLK, SQRTDEG, EPSDEG, WnT_ext, b_ext):
    return [
        {
            "x": xs_pad,
            "xself": np.ascontiguousarray(XSELF[i]),
            "idx": np.ascontiguousarray(IDX[i]),
            "idx0": np.ascontiguousarray(IDX[i][:, : _N0 * 8]),
            "s": np.ascontiguousarray(SBLK[i].reshape(P, -1)),
            "sqrtdeg": np.ascontiguousarray(SQRTDEG[i]),
            "epsdeg": np.ascontiguousarray(EPSDEG[i]),
            "wnT": WnT_ext,
            "b": b_ext,
        }
        for i in range(NCORES)
    ]


def kernel(x, edge_index, W, b, u):
    pre = _preprocess(x, edge_index, W, b, u)
    nc = _build(pre[-1])
    in_maps = _make_in_maps(*pre[:-1])

    # The axon terminal can be transiently unavailable right after a prior
    # process's teardown; retry with backoff.
    import time

    last_err = None
    for attempt in range(6):
        try:
            res = run_bass_kernel_spmd(nc, in_maps, list(range(NCORES)))
            break
        except Exception as e:  # noqa: BLE001
            last_err = e
            time.sleep(45)
    else:
        raise last_err
    shards = [np.asarray(res.results[i]["out"]) for i in range(NCORES)]
    out = np.concatenate(shards, axis=0)[:N]
    return out.astype(np.float32)
